# revision 11
# baseline (speedup 1.0000x reference)
"""Trainium2 Bass kernel for fused multi-head causal attention (GPT-2 style).

Full-input contract: kernel(**inputs) takes the complete tensors and returns
the complete output. Internally: data-parallel over the batch dim (B=8) across
8 NeuronCores; each core runs the whole attention block for one batch element.
The c_proj bias is added on the host after the gather (free vs device time).

Per-core dataflow (S=512, D=1024, H=16, dh=64). All matmul operands are bf16
(PSUM accumulation stays fp32), which removes the fp32r small-free-dim penalty,
enables fast-weight-load, and halves DMA + SBUF traffic. x^T, W1's V-columns
and W2 are host-packed partition-major so each lands as one contiguous DMA
with 16KB per-partition lines:

  x^T [D,S]  bf16 (host-packed, resident in SBUF)
  V:   psum[s,n]   = x^T[:,s].T @ W1v            -> [V|1]/[1|V] head blocks
  QK:  psum[n,s]   = W1[:,n].T @ x^T             (n-tiles on partitions -> Q^T,K^T)
  S^T: psum[sk,sq] = K_h^T[:,sk].T @ Q_h^T       (scores transposed, causal-trimmed)
       + idn @ trilneg on the 128-wide diagonal block only (causal -1e9)
  P^T  = exp(S^T/8 + pad_bias)  (ACT, bf16 out)
  A^T: psum = [V_h|1].T @ P^T                    (64 attn rows + 64 denominator rows)
  A^T_norm = psum_attn * recip_approx(psum_denom)  -> A^T tiles [n, s] bf16
  out: psum[s,e]   = A^T[:,s].T @ W2             (fp32 out)

Engine balance: PE ~80us of matmul stream (the bottleneck), exps + Q/K bias
adds on ACT (whose queue carries no big DMA descriptors), denominator staging
+ reciprocal_approx_fast + normalize + V-bias/out copies on DVE, ones-block
memset on GpSimd. DMA queues: sync carries msk/x^T/per-pass QK weights, gpsimd
carries W1v early and W2 mid-pipeline, scalar only small consts; output tiles
alternate across the three queues to shorten the tail. Heads processed in
pairs (even head on partitions 0:64, odd on 64:128) with pair i's score
matmuls followed by pass i+1's QK projection (independent PE work that hides
pair i's ACT exps) before pair i's A^T matmuls.
"""

import sys

if "/opt/trn_rl_repo" not in sys.path:
    sys.path.insert(0, "/opt/trn_rl_repo")

import numpy as np

import concourse.bass as bass
import concourse.mybir as mybir
import concourse.tile as tile
from concourse import bacc
from concourse.bass_utils import run_bass_kernel_spmd
from concourse.masks import make_identity

F32 = mybir.dt.float32
BF16 = mybir.dt.bfloat16
NP_BF16 = mybir.dt.np(BF16)

B, S, D = 8, 512, 1024
H = 16
DH = D // H          # 64
NT_S = S // 128      # 4 s-tiles
ND = D // 128        # 8 d-tiles
N_CORES = 8
SCALE = 1.0 / 8.0    # 1/sqrt(head_dim)

_CACHED = {}
VARIANT = {}


def _dram_ap(t, offset, dims):
    """Raw strided DRAM access pattern ([step, count] pairs, elements)."""
    return bass.AP(tensor=t[...].tensor if hasattr(t, "shape") else t.tensor,
                   offset=offset, ap=dims)


def _sb_ap(tile_ap, offset, free_dims):
    """Raw strided SBUF access pattern reusing the tile's partition dim."""
    return bass.AP(tensor=tile_ap.tensor, offset=tile_ap.offset + offset,
                   ap=[tile_ap.ap[0]] + free_dims)


def _emit_body(nc, tc, pools, dram, consts, phases=("v", "qk", "attn", "cproj")):
    (xt_p, w1v_p, w1qk_p, qkt_p, vsb_p, pt_p, recip_p, at_p, w2sb_p, out_p,
     mmps_p, scps_p, avps_p) = pools
    xt, msk, w1vp, w1qk_pk, b1, w2p, out = dram
    idn, trilneg, pad_bias, bqk, bv = consts

    do_v = "v" in phases
    do_qk = "qk" in phases
    do_attn = "attn" in phases
    do_cproj = "cproj" in phases

    # -------- resident x^T (chunked per d so QK pass 0 starts early) -------
    # Startup is HBM-bound (all 8 cores pull weights at once), and the PE
    # executes its queue in order — so emission order tracks data arrival:
    # QK pass 0 (x^T + 0.5MB of weights) first, V c=0 next, V c=1 deferred
    # into the pair loop.
    xt_sb = xt_p.tile([128, ND, S], BF16, tag="xt")
    for d in range(ND):
        nc.sync.dma_start(xt_sb[:, d, :],
                          _dram_ap(xt, d * S, [[ND * S, 128], [1, S]]))
    w1v = w1v_p.tile([128, 2, ND, 512], BF16, tag="w1v")
    w2sb = w2sb_p.tile([128, ND, D], BF16, tag="w2sb")

    # ---------------- V projection into [V|1]/[1|V] head blocks ----------
    va_sb = vsb_p.tile([128, NT_S, H, 128], BF16, tag="va")
    va = va_sb[:]
    if do_v:
        # ones half-blocks: for pair p, flat cols [256p+64, 256p+192), all t
        nc.gpsimd.memset(
            _sb_ap(va, 64, [[2048, NT_S], [256, 8], [1, 128]]), 1.0
        )

    def emit_v_half(c):
        for t in range(NT_S):
            ps = mmps_p.tile([128, 512], F32, tag="mmps")
            for d in range(ND):
                nc.tensor.matmul(
                    ps[:],
                    xt_sb[:, d, t * 128 : (t + 1) * 128],
                    w1v[:, c, d, :],
                    start=(d == 0), stop=(d == ND - 1),
                )
            # scatter 8 heads' 64-col chunks into their [V|1] blocks in one op:
            # head h=8c+2u+v gets flat offset 2048t+1024c+256u+192v
            nc.vector.tensor_tensor(
                out=_sb_ap(va, t * 2048 + c * 1024, [[256, 4], [192, 2], [1, 64]]),
                in0=ps[:], in1=bv[:, c * 512 : (c + 1) * 512],
                op=mybir.AluOpType.add,
            )

    # ------------- QK projection pass i -> attention for heads 2i, 2i+1 ----
    at_sb = at_p.tile([128, ND, S], BF16, tag="at")

    def emit_qk_pass(i, dma_eng=None):
        # w1 columns {128i..128i+128} (Q) and {D+128i..} (K) for all 8 d-tiles
        wqk = w1qk_p.tile([128, ND, 2, 128], BF16, tag="w1qk")
        (dma_eng or nc.sync).dma_start(wqk[:], w1qk_pk[i])
        if not do_qk:
            return None, None
        psq = mmps_p.tile([128, 512], F32, tag="mmps")
        psk = mmps_p.tile([128, 512], F32, tag="mmps")
        for d in range(ND):
            nc.tensor.matmul(psq[:], wqk[:, d, 0, :], xt_sb[:, d, :],
                             start=(d == 0), stop=(d == ND - 1))
            nc.tensor.matmul(psk[:], wqk[:, d, 1, :], xt_sb[:, d, :],
                             start=(d == 0), stop=(d == ND - 1))
        qt = qkt_p.tile([128, S], BF16, tag="qkt")
        kt = qkt_p.tile([128, S], BF16, tag="qkt")
        nc.scalar.add(qt[:], psq[:], bqk[:, i : i + 1])
        nc.scalar.add(kt[:], psk[:], bqk[:, ND + i : ND + i + 1])
        return qt, kt

    # startup order: wqk0 on the (idle) scalar queue in parallel with x^T on
    # sync and W1v-c0 on gpsimd; QK pass 0 computes first, then V c=0.
    qt, kt = emit_qk_pass(0, dma_eng=nc.scalar)
    nc.gpsimd.dma_start(w1v[:, 0], _dram_ap(w1vp, 0, [[2 * ND * 512, 128], [1, ND * 512]]))
    # V-bias broadcast after wqk0 on the scalar queue (needed ~V c0 bias time)
    nc.scalar.dma_start(out=bv[:], in_=b1[None, 2 * D : 3 * D].to_broadcast([128, D]))
    if do_v:
        emit_v_half(0)
    nc.gpsimd.dma_start(w1v[:, 1],
                        _dram_ap(w1vp, ND * 512, [[2 * ND * 512, 128], [1, ND * 512]]))
    for i in range(ND):
        if i == 1 and do_v:
            # V c=1 feeds pairs 4-7; its weights have landed by now
            emit_v_half(1)
        if i == 2 and do_cproj:
            # W2 lands mid-pipeline, long before c_proj needs it
            nc.gpsimd.dma_start(w2sb[:], w2p[:])
        if not do_attn:
            if i + 1 < ND:
                qt, kt = emit_qk_pass(i + 1)
            continue
        h_e, h_o = 2 * i, 2 * i + 1
        av_e = avps_p.tile([128, 512], F32, tag="avps")
        av_o = avps_p.tile([128, 512], F32, tag="avps")
        # --- scores for all sk (feeds ACT early) ---
        pts = []
        for sk in range(NT_S):
            w = S - sk * 128
            # both heads' scores in one 2-bank tile -> single paired exp
            sc = scps_p.tile([128, 2, 512], F32, tag="scps")
            nc.tensor.matmul(sc[:, 0, 0:w], kt[0:64, sk * 128 : (sk + 1) * 128],
                             qt[0:64, sk * 128 : S], start=True, stop=False)
            nc.tensor.matmul(sc[:, 1, 0:w], kt[64:128, sk * 128 : (sk + 1) * 128],
                             qt[64:128, sk * 128 : S], start=True, stop=False)
            # causal -1e9 needed only on the 128-wide diagonal block
            nc.tensor.matmul(sc[:, 0, 0:128], idn[:], trilneg[:],
                             start=False, stop=True)
            nc.tensor.matmul(sc[:, 1, 0:128], idn[:], trilneg[:],
                             start=False, stop=True)
            pt = pt_p.tile([128, 2, 512], BF16, tag="pt")
            nc.scalar.activation(pt[:, :, 0:w], sc[:, :, 0:w],
                                 mybir.ActivationFunctionType.Exp,
                                 bias=pad_bias[:, sk : sk + 1], scale=SCALE)
            pts.append(pt)
        # --- next pass's QK projection: PE work independent of the exps ---
        if i + 1 < ND:
            qt_n, kt_n = emit_qk_pass(i + 1)
        # --- A^T accumulation (exps are done or nearly done by now) ---
        for sk in range(NT_S):
            w = S - sk * 128
            for j, (h, av) in enumerate(((h_e, av_e), (h_o, av_o))):
                nc.tensor.matmul(av[:, sk * 128 : S], va_sb[:, sk, h, :],
                                 pts[sk][:, j, 0:w],
                                 start=(sk == 0), stop=(sk == NT_S - 1))
        # normalize: attn rows * approx-recip(denom rows). The custom DVE
        # reciprocal needs a full-tile partition-aligned SBUF operand, so
        # stage both heads' denominators into one [128,512] tile first.
        dsb = recip_p.tile([128, 512], F32, tag="recip")
        nc.vector.tensor_copy(dsb[0:64, :], av_e[64:128, :])
        nc.vector.tensor_copy(dsb[64:128, :], av_o[0:64, :])
        rc = recip_p.tile([128, 512], F32, tag="recip")
        nc.vector.reciprocal_approx_fast(rc[:], dsb[:])
        nc.vector.tensor_tensor(out=at_sb[0:64, i, :], in0=av_e[0:64, :],
                                in1=rc[0:64, :], op=mybir.AluOpType.mult)
        nc.vector.tensor_tensor(out=at_sb[64:128, i, :], in0=av_o[64:128, :],
                                in1=rc[64:128, :], op=mybir.AluOpType.mult)
        if i + 1 < ND:
            qt, kt = qt_n, kt_n

    # ---------------- c_proj (bias added on host) ----------------
    out_qs = [nc.scalar, nc.sync, nc.gpsimd]
    for t in range(NT_S) if do_cproj else []:
        for c in range(2):
            ps = mmps_p.tile([128, 512], F32, tag="mmps")
            for d in range(ND):
                nc.tensor.matmul(
                    ps[:],
                    at_sb[:, d, t * 128 : (t + 1) * 128],
                    w2sb[:, d, c * 512 : (c + 1) * 512],
                    start=(d == 0), stop=(d == ND - 1),
                )
            ob = out_p.tile([128, 512], F32, tag="outsb")
            nc.vector.tensor_copy(ob[:], ps[:])
            out_qs[(2 * t + c) % 3].dma_start(
                out[t * 128 : (t + 1) * 128, c * 512 : (c + 1) * 512], ob[:]
            )


def _build_nc(repeats=1, loop_n=None, phases=("v", "qk", "attn", "cproj")):
    nc = bacc.Bacc("TRN2", target_bir_lowering=False, debug=False)

    # x^T, W1v, W2 are host-packed partition-major: [128, n] with each
    # partition's data contiguous in DRAM (16KB lines -> fast DMA)
    xt = nc.dram_tensor("xt", [128, ND * S], BF16, kind="ExternalInput")
    msk = nc.dram_tensor("msk", [S], F32, kind="ExternalInput")
    w1vp = nc.dram_tensor("w1vp", [128, 2 * ND * 512], BF16, kind="ExternalInput")
    w1qk_pk = nc.dram_tensor("w1qk_pk", [ND, 128, ND * 2 * 128], BF16,
                             kind="ExternalInput")  # host-packed QK slices
    b1 = nc.dram_tensor("b1", [3 * D], F32, kind="ExternalInput")
    w2p = nc.dram_tensor("w2p", [128, ND * D], BF16, kind="ExternalInput")
    out = nc.dram_tensor("out", [S, D], F32, kind="ExternalOutput")
    dram = (xt, msk, w1vp, w1qk_pk, b1, w2p, out)

    with tile.TileContext(nc) as tc:
        with (
            tc.tile_pool(name="const", bufs=1) as const_p,
            tc.tile_pool(name="xt", bufs=1) as xt_p,
            tc.tile_pool(name="w1v", bufs=1) as w1v_p,
            tc.tile_pool(name="w1qk", bufs=3) as w1qk_p,
            tc.tile_pool(name="qkt", bufs=4) as qkt_p,
            tc.tile_pool(name="vsb", bufs=1) as vsb_p,
            tc.tile_pool(name="pt", bufs=5) as pt_p,
            tc.tile_pool(name="recip", bufs=4) as recip_p,
            tc.tile_pool(name="at", bufs=1) as at_p,
            tc.tile_pool(name="w2sb", bufs=1) as w2sb_p,
            tc.tile_pool(name="outsb", bufs=2) as out_p,
            tc.tile_pool(name="mmps", bufs=2, space="PSUM") as mmps_p,
            tc.tile_pool(name="scps", bufs=2, space="PSUM") as scps_p,
            tc.tile_pool(name="avps", bufs=2, space="PSUM") as avps_p,
        ):
            # ---- constants (once) ----
            idn = const_p.tile([128, 128], BF16)
            make_identity(nc, idn[:])
            # trilneg[p, c] = -1e9 where c < p else 0 (causal diag block)
            trilneg = const_p.tile([128, 128], BF16)
            nc.gpsimd.memset(trilneg[:], 0.0)
            nc.gpsimd.affine_select(
                out=trilneg[:], in_=trilneg[:],
                compare_op=mybir.AluOpType.is_ge, fill=-1e9, base=0,
                pattern=[[1, 128]], channel_multiplier=-1,
            )

            msk_sb = const_p.tile([128, NT_S], F32)
            nc.sync.dma_start(msk_sb[:], _dram_ap(msk, 0, [[1, 128], [128, NT_S]]))
            pad_bias = const_p.tile([128, NT_S], F32)
            nc.vector.tensor_scalar(
                out=pad_bias[:], in0=msk_sb[:], scalar1=1.0, scalar2=1e9,
                op0=mybir.AluOpType.subtract, op1=mybir.AluOpType.mult,
            )

            bqk = const_p.tile([128, 2 * ND], F32)
            nc.scalar.dma_start(bqk[:], _dram_ap(b1, 0, [[1, 128], [128, 2 * ND]]))
            bv = const_p.tile([128, D], F32)

            consts = (idn, trilneg, pad_bias, bqk, bv)
            pools = (xt_p, w1v_p, w1qk_p, qkt_p, vsb_p, pt_p, recip_p, at_p,
                     w2sb_p, out_p, mmps_p, scps_p, avps_p)
            if loop_n is not None:
                with tc.For_i(0, loop_n, 1):
                    _emit_body(nc, tc, pools, dram, consts, phases)
            else:
                for _ in range(repeats):
                    _emit_body(nc, tc, pools, dram, consts, phases)

    nc.compile()
    return nc


def _get_nc(repeats=1, loop_n=None, phases=("v", "qk", "attn", "cproj")):
    key = ("nc", repeats, loop_n, tuple(phases), tuple(sorted(VARIANT.items())))
    if key not in _CACHED:
        _CACHED[key] = _build_nc(repeats, loop_n, phases)
    return _CACHED[key]


def _pack_w1qk(w1):
    # pack per-pass QK weight slices: pass i needs w1[:, 128i:128i+128] (Q)
    # and w1[:, D+128i:D+128i+128] (K) for each of the 8 d-tiles, laid out
    # [pass, partition, d, {q,k}, col] so each pass is one contiguous DMA
    w1r = w1.reshape(ND, 128, 3 * D)
    qs = w1r[:, :, :D].reshape(ND, 128, ND, 128)       # [d, p, i, c]
    ks = w1r[:, :, D:2 * D].reshape(ND, 128, ND, 128)
    pk = np.stack([qs, ks], axis=3)                    # [d, p, i, {q,k}, c]
    return np.ascontiguousarray(pk.transpose(2, 1, 0, 3, 4).reshape(
        ND, 128, ND * 2 * 128))


def _make_in_maps(inputs):
    x = np.asarray(inputs["x"], dtype=np.float32)
    mask = np.asarray(inputs["mask"], dtype=np.float32)
    w1 = np.asarray(inputs["c_attn_w"], dtype=np.float32).astype(NP_BF16)
    b1 = np.ascontiguousarray(np.asarray(inputs["c_attn_b"], dtype=np.float32))
    w2 = np.asarray(inputs["c_proj_w"], dtype=np.float32).astype(NP_BF16)
    w1qk_pk = _pack_w1qk(w1)
    # partition-major packs: [p, ...] with per-partition data contiguous
    w1vp = np.ascontiguousarray(
        w1[:, 2 * D:].reshape(ND, 128, 2, 512).transpose(1, 2, 0, 3)
        .reshape(128, 2 * ND * 512))
    w2p = np.ascontiguousarray(
        w2.reshape(ND, 128, D).transpose(1, 0, 2).reshape(128, ND * D))
    xt_all = x.transpose(0, 2, 1).astype(NP_BF16)      # [B, D, S]
    in_maps = []
    for b in range(N_CORES):
        xtp = np.ascontiguousarray(
            xt_all[b].reshape(ND, 128, S).transpose(1, 0, 2).reshape(128, ND * S))
        in_maps.append({
            "xt": xtp,
            "msk": np.ascontiguousarray(mask[b]),
            "w1vp": w1vp, "w1qk_pk": w1qk_pk, "b1": b1, "w2p": w2p,
        })
    return in_maps


def kernel(x, mask, c_attn_w, c_attn_b, c_proj_w, c_proj_b):
    nc = _get_nc()
    in_maps = _make_in_maps(dict(x=x, mask=mask, c_attn_w=c_attn_w,
                                 c_attn_b=c_attn_b, c_proj_w=c_proj_w,
                                 c_proj_b=c_proj_b))
    res = run_bass_kernel_spmd(nc, in_maps, list(range(N_CORES)))
    b2 = np.asarray(c_proj_b, dtype=np.float32)
    return np.stack([res.results[b]["out"] for b in range(N_CORES)], axis=0) + b2


if __name__ == "__main__":
    pass
